# revision 29
# baseline (speedup 1.0000x reference)
"""Data-parallel 3x3 conv2d (stride 1, pad 1) on 8 Trainium2 NeuronCores.

Problem: x [32, 64, 112, 112] f32, weight [128, 64, 3, 3] f32, bias [128]
-> out [32, 128, 112, 112] f32.

Sharding: batch N=32 split 4 images per core across 8 cores; weight/bias
replicated (forward only, no collectives).

Per-core kernel (Bass/Tile, implicit GEMM, fp16 compute / fp32 accumulate):
  - Two images are processed concurrently: image pair (2p, 2p+1) lives in
    one SBUF tile [128, 114*113+1] f16 — partitions 0-63 hold image 2p's
    64 channels in a shared-pad layout (row stride 113: each row's right
    pad IS the next row's left pad, both zero), partitions 64-127 hold
    image 2p+1.  All 9 conv taps are flat column offsets kh*113+kw.
  - Each output tile = 4 output rows = 451 moving columns (1 junk col
    per row).  Per tile, 9 K=64 matmuls accumulate into a PSUM bank.
    The two images' matmuls are interleaved A,B,A,B: they land on PE
    row-tiles T0/T8 (64x128 tiling mode, auto-derived from the APs' base
    partitions) and execute CONCURRENTLY -> 4.5 effective passes per
    tile instead of 6 with the K=128 kh-pairing scheme, and no input
    duplication in SBUF or on the DMA path.  Measured slot = 193 ns vs
    451/2.4GHz = 188 ns floor; MM starts are pc-monotone so A/B MUST be
    interleaved (chain-after-chain would serialize the row tiles).
  - fp16 halves DMA traffic vs fp32 (19.4 MB vs 51.8 MB per core, which
    was ~94% DMA-occupancy in the fp32r baseline) and enables Fast
    Weight Load so the per-matmul LDWEIGHTS hides under the moving
    stream.  Accumulation stays fp32 in PSUM (rel err 3.4e-4).
  - Loads: graduated contiguous chunk DMAs (9..40 x-rows) split across
    both HWDGE queues, then one DVE scatter per chunk into the padded
    layout (direct strided DMA into the layout measured ~4x slower —
    descriptor-bound at 224 B/row).  Pad cells are zeroed once per
    buffer.  13 dep-free warm-up matmul pairs on a memset tile bridge
    the Tile preamble (~7 us) + first-chunk latency (~5.5 us: trigger
    0.8 + transfer + ~2.5 us completion receipt + scatter) and flip the
    PE HAM clock gate to 8/8 before real work.  Pair 1 loads issue at
    tile 4 of pair 0's compute so their HWDGE triggers can't head-of-
    line block PSUM drains.
  - Epilogue: ScalarE activation(Identity, bias) drains each PSUM bank
    contiguously to fp16 SBUF (the last pair's final two B drains go to
    DVE so the tail's A/B drains run in parallel); 7-tile-batched
    contiguous stores, image-A on the ACT queue, image-B on the SP
    queue, with the final batch split 4+2+1 to shorten the drain tail.
    Junk cols are stripped on the host after gathering.
"""
import sys

if '/opt/trn_rl_repo' not in sys.path:
    sys.path.insert(0, '/opt/trn_rl_repo')

import numpy as np

N, CIN, HH, WW = 32, 64, 112, 112
OC = 128
NCORES = 8
N_PER_CORE = N // NCORES
NPAIR = N_PER_CORE // 2          # image pairs per core

HP = HH + 2                      # 114 padded rows
XW = HH + 1                      # 113: shared-pad row stride -- each row's
                                 # right pad IS the next row's left pad
                                 # (both structurally zero)
FLAT = HP * XW                   # 12882
XCOLS = FLAT + 1                 # tap (2,2) of the last tile reads 1 past
RPT = 4                          # output rows per PSUM tile
NCOL = RPT * XW                  # 452 flat cols per tile (f0 stride)
MVN = NCOL - 1                   # 451 moving columns per matmul (1 junk
                                 # col per row at wo=112)
NT = HH // RPT                   # 28 tiles per image
TAP_OFF = [kh * XW + kw for kh in range(3) for kw in range(3)]
STB = 7                          # tiles per batched store
YCOLS = NT * MVN                 # 12628 stored cols per image (with junk)

_cache = {}


def _build():
    import concourse.bacc as bacc
    import concourse.mybir as mybir
    from concourse.tile import TileContext

    F32 = mybir.dt.float32
    F16 = mybir.dt.float16

    nc = bacc.Bacc("TRN2", target_bir_lowering=False, debug=False,
                   num_devices=NCORES)
    # x packed on host as [(n c), (h w)] fp16 so chunk loads are plain 2D
    # slices with a 128-partition dim (2 images per 128 partitions).
    x = nc.declare_dram_parameter("x", [N_PER_CORE * CIN, HH * WW], F16,
                                  isOutput=False)
    wt = nc.declare_dram_parameter("wt", [128, 9 * 128], F16, isOutput=False)
    bias = nc.declare_dram_parameter("bias", [128, 1], F32, isOutput=False)
    y = nc.declare_dram_parameter("y", [N_PER_CORE, OC, YCOLS], F16,
                                  isOutput=True)
    xa = x.ap()
    ya = y.ap()

    with TileContext(nc) as tc:
        with (
            tc.tile_pool(name="wpool", bufs=1) as wpool,
            tc.tile_pool(name="stgpool", bufs=4) as stgpool,
            tc.tile_pool(name="xpool", bufs=1) as xpool,
            tc.tile_pool(name="opool", bufs=2) as opool,
            tc.tile_pool(name="pspool", bufs=4, space="PSUM") as pspool,
        ):
            # wt gates the first real matmul: it goes FIRST on the ACT
            # HWDGE queue (x chunks are first on the SP queue; queueing
            # bias ahead of wt was measured to delay the wt semaphore by
            # ~2 us).  bias only gates the first PSUM drain, which has
            # bank slack — load it on the independent SWDGE queue.
            wtile = wpool.tile([128, 9 * 128], F16, tag="w")
            nc.scalar.dma_start(out=wtile[:, :], in_=wt[:, :])
            btile = wpool.tile([128, 1], F32, tag="b")
            nc.gpsimd.dma_start(out=btile[:, :], in_=bias[:, :])
            # memset-fed warm-up weights: no DMA dependency, so the PE can
            # start its HAM warm-up right after the Tile preamble.  10 cold
            # ~427ns slots bridge to the first chunk's usable time.
            wme = wpool.tile([128, 512], F16, tag="wme")
            nc.gpsimd.memset(wme[:, :], 0.0)

            xts = [xpool.tile([128, XCOLS], F16, tag=f"x{i}", name=f"xt{i}")
                   for i in range(NPAIR)]
            # zero the pad borders once per buffer; chunk scatters write
            # only interior pixels.
            for xt in xts:
                # xpad row 0 plus (row 1, col 0)
                nc.vector.memset(xt[:, 0:XW + 1], 0.0)
                # col 0 of rows 1..112 (shared left/right pads)
                nc.vector.memset(
                    xt[:, XW: XW + 112 * XW]
                      .rearrange("p (r t) -> p r t", r=112, t=XW)[:, :, 0:1],
                    0.0)
                # xpad row 113 plus the overrun col
                nc.vector.memset(xt[:, (HP - 1) * XW: XCOLS], 0.0)

            # HAM warm-up: dep-free 64x128-tile matmul pairs keep the PE
            # busy until the first x chunk lands (~2 cold slots each).
            for _ in range(10):
                pswa = pspool.tile([128, 512], F32, tag="psA", name="pswa")
                pswb = pspool.tile([128, 512], F32, tag="psB", name="pswb")
                nc.tensor.matmul(pswa[:, :], wme[0:64, 0:128],
                                 wme[0:64, 0:512], start=True, stop=True)
                nc.tensor.matmul(pswb[:, :], wme[64:128, 0:128],
                                 wme[64:128, 0:512], start=True, stop=True)

            def load_pair(p, chunk_rows, engines):
                xt = xts[p]
                xt3 = xt[:, 0:FLAT].rearrange("p (r c) -> p r c", r=HP, c=XW)
                r0 = 0
                for nr, eng in zip(chunk_rows, engines):
                    stg = stgpool.tile([128, 40 * WW], F16, tag="stg")
                    eng.dma_start(
                        out=stg[:, 0:nr * WW],
                        in_=xa[p * 128:(p + 1) * 128,
                               r0 * WW:(r0 + nr) * WW])
                    nc.vector.tensor_copy(
                        xt3[:, 1 + r0:1 + r0 + nr, 1:1 + WW],
                        stg[:, 0:nr * WW].rearrange("p (r c) -> p r c",
                                                    r=nr, c=WW))
                    r0 += nr

            def compute_pair(p, prefetch=None):
                xt = xts[p]
                otA = otB = None
                for t in range(NT):
                    if t == 4 and prefetch is not None:
                        # issue the next pair's loads here: late enough
                        # that their HWDGE triggers can't head-of-line
                        # block the early PSUM drains / store triggers.
                        prefetch()
                    f0 = t * NCOL
                    psA = pspool.tile([128, MVN], F32, tag="psA")
                    psB = pspool.tile([128, MVN], F32, tag="psB")
                    for s in range(9):
                        o = f0 + TAP_OFF[s]
                        nc.tensor.matmul(
                            psA[:, :], wtile[0:64, s * 128:(s + 1) * 128],
                            xt[0:64, o:o + MVN],
                            start=(s == 0), stop=(s == 8),
                            skip_group_check=True)
                        nc.tensor.matmul(
                            psB[:, :], wtile[64:128, s * 128:(s + 1) * 128],
                            xt[64:128, o:o + MVN],
                            start=(s == 0), stop=(s == 8),
                            skip_group_check=True)
                    if t % STB == 0:
                        otA = opool.tile([128, STB * MVN], F16, tag="oA")
                        otB = opool.tile([128, STB * MVN], F16, tag="oB")
                    sl = slice((t % STB) * MVN, (t % STB + 1) * MVN)
                    nc.scalar.activation(
                        otA[:, sl], psA[:, :],
                        mybir.ActivationFunctionType.Identity, bias=btile[:, :])
                    # the last pair's final two B drains go to DVE so the
                    # kernel tail's A/B drains run in parallel.
                    if p == NPAIR - 1 and t >= NT - 2:
                        nc.vector.tensor_scalar_add(otB[:, sl], psB[:, :],
                                                    btile[:, :])
                    else:
                        nc.scalar.activation(
                            otB[:, sl], psB[:, :],
                            mybir.ActivationFunctionType.Identity,
                            bias=btile[:, :])
                    # A stores trigger on the ACT queue, B stores on the SP
                    # queue (idle after loads) so the triggers overlap.  The
                    # final batch is split 4+3 to shorten the drain tail.
                    last = (p == NPAIR - 1 and t == NT - 1)
                    if t % STB == STB - 1 and not last:
                        g = slice((t - STB + 1) * MVN, (t + 1) * MVN)
                        nc.scalar.dma_start(out=ya[2 * p, :, g],
                                            in_=otA[:, :])
                        nc.sync.dma_start(out=ya[2 * p + 1, :, g],
                                          in_=otB[:, :])
                    elif p == NPAIR - 1 and t in (NT - 4, NT - 2, NT - 1):
                        # finer trailing stores so the drain tail is short
                        lo = {NT - 4: NT - STB, NT - 2: NT - 3,
                              NT - 1: NT - 1}[t]
                        g1 = slice(lo * MVN, (t + 1) * MVN)
                        o1 = slice((lo - (NT - STB)) * MVN,
                                   (t + 1 - (NT - STB)) * MVN)
                        nc.scalar.dma_start(out=ya[2 * p, :, g1],
                                            in_=otA[:, o1])
                        nc.sync.dma_start(out=ya[2 * p + 1, :, g1],
                                          in_=otB[:, o1])

            # pair 0 loads in graduated chunks split across both HWDGE
            # queues so compute starts as early as possible and each
            # chunk lands before the PE reaches its rows.
            load_pair(0, [9, 14, 21, 28, 40],
                      [nc.sync, nc.scalar, nc.sync, nc.scalar, nc.sync])
            compute_pair(0, prefetch=lambda: load_pair(
                1, [28, 28, 28, 28], [nc.sync] * 4))
            compute_pair(1)
    nc.compile()
    return nc


def _pack_weights(weight: np.ndarray) -> np.ndarray:
    """[O=128, C=64, 3, 3] -> [128, 9*128] f16: rows 0-63 and 64-127 both
    hold slab s=(kh*3+kw) at cols [s*128,(s+1)*128) with [c, o] layout."""
    w9 = np.transpose(weight.astype(np.float32), (1, 2, 3, 0)).reshape(64, 9 * 128)
    return np.ascontiguousarray(
        np.concatenate([w9, w9], axis=0).astype(np.float16))


def kernel(x: np.ndarray, weight: np.ndarray, bias: np.ndarray,
           _trace: bool = False) -> np.ndarray:
    from concourse.bass_utils import run_bass_kernel_spmd

    x = np.asarray(x, dtype=np.float32)
    weight = np.asarray(weight, dtype=np.float32)
    bias = np.asarray(bias, dtype=np.float32)
    assert x.shape == (N, CIN, HH, WW), x.shape
    assert weight.shape == (OC, CIN, 3, 3), weight.shape
    assert bias.shape == (OC,), bias.shape

    if 'nc' not in _cache:
        _cache['nc'] = _build()
    nc = _cache['nc']

    x16 = np.ascontiguousarray(
        x.reshape(NCORES, N_PER_CORE * CIN, HH * WW).astype(np.float16))
    wtp = _pack_weights(weight)
    bp = np.ascontiguousarray(bias.reshape(128, 1).astype(np.float32))
    in_maps = [{"x": x16[i], "wt": wtp, "bias": bp} for i in range(NCORES)]
    res = run_bass_kernel_spmd(nc, in_maps, core_ids=list(range(NCORES)),
                               trace=_trace)
    # y: [4, 128, 28*456] f16 per core; strip the 2 junk cols per 114 and
    # upcast on the host.
    out = np.empty((N, OC, HH, WW), np.float32)
    for i in range(NCORES):
        yc = res.results[i]["y"].reshape(N_PER_CORE, OC, NT, MVN)
        yc = np.concatenate(
            [yc, np.zeros((N_PER_CORE, OC, NT, NCOL - MVN), yc.dtype)],
            axis=-1).reshape(N_PER_CORE, OC, NT, RPT, XW)
        out[N_PER_CORE * i: N_PER_CORE * (i + 1)] = (
            yc[..., :WW].astype(np.float32).reshape(N_PER_CORE, OC, HH, WW))
    if _trace:
        _cache['last_exec_time_ns'] = res.exec_time_ns
    return out
